# revision 1
# baseline (speedup 1.0000x reference)
import numpy as np

# Problem constants (hardcoded per contract: kernel.py is self-contained)
B, S, IDIM, D, T = 32, 512, 256, 256, 4096
EPS = 1e-6
SIGMA_C = 2.0
PAD = 0
LOG_SQRT_2PI = 0.9189385332046727  # 0.5*log(2*pi)
N_CORES = 8
S2 = S + 2


def _compute(text_p, durs_p, embed, total_time, xp):
    """Core math on a [b, S+2] shard; xp is jnp or np."""
    cum = xp.cumsum(durs_p, axis=-1)
    durs_f = durs_p.astype(xp.float32)
    c = durs_f / 2.0 + (cum - durs_p).astype(xp.float32)
    sig = durs_f / SIGMA_C + EPS

    t = xp.arange(total_time, dtype=xp.float32) + 0.5
    z = (t[None, :, None] - c[:, None, :]) / sig[:, None, :]
    w = xp.exp(-0.5 * z * z - xp.log(sig)[:, None, :] - LOG_SQRT_2PI)

    token_pad = text_p == PAD
    w = xp.where(token_pad[:, None, :], 0.0, w)
    w = w / (w.sum(-1, keepdims=True) + EPS)

    time_invalid = xp.arange(total_time)[None, :] >= cum[:, -1:]
    w = xp.where(time_invalid[:, :, None], 0.0, w)
    last_col = xp.arange(S2) == S2 - 1
    w = xp.where(time_invalid[:, :, None] & last_col[None, None, :], 1.0, w)

    emb = embed[text_p]
    return xp.einsum("bts,bsd->btd", w, emb)


def kernel(text, durs, embed, total_time):
    text = np.asarray(text)
    durs = np.asarray(durs)
    embed = np.asarray(embed, dtype=np.float32)
    tt = int(np.asarray(total_time))

    # F.pad(text/durs, [0, 2]) with value 0; int32 is enough (T <= 4096 fits)
    text_p = np.pad(text, ((0, 0), (0, 2)), constant_values=PAD).astype(np.int32)
    durs_p = np.pad(durs, ((0, 0), (0, 2)), constant_values=0).astype(np.int32)

    try:
        import jax
        import jax.numpy as jnp

        devs = jax.devices()
        if len(devs) < N_CORES:
            raise RuntimeError("need 8 cores")
        b_per = B // N_CORES  # 4 rows per core, pure data parallel

        def per_core(tp, dp, emb):
            return _compute(tp, dp, emb, tt, jnp)

        pm = jax.pmap(per_core, devices=devs[:N_CORES])
        tp_sh = text_p.reshape(N_CORES, b_per, S2)
        dp_sh = durs_p.reshape(N_CORES, b_per, S2)
        emb_rep = np.broadcast_to(embed, (N_CORES, IDIM, D))
        out = pm(tp_sh, dp_sh, emb_rep)  # [8, 4, T, D]
        out = np.asarray(out).reshape(B, tt, D)
        return out.astype(np.float32)
    except Exception:
        # CPU fallback: identical math, chunked over batch to bound memory
        out = np.empty((B, tt, D), dtype=np.float32)
        for i in range(0, B, 4):
            out[i : i + 4] = _compute(
                text_p[i : i + 4], durs_p[i : i + 4], embed, tt, np
            )
        return out

